# revision 25
# baseline (speedup 1.0000x reference)
"""GNN message-passing (ConvGraph) Trainium2 Bass kernel, 8 NeuronCores.

Computes out = segment_sum(edge_weight * (x @ W)[edge_src], edge_dst) for a
graph with N nodes and E edges.

Strategy (v2 — chunk-major pipeline, no collective):
  - Shard OUTPUT nodes (rows of out) across the 8 cores; replicate W and x.
  - Everything except PSUM runs in bf16 (tolerance 2e-2; we are ~4e-3).
  - The per-edge h-row gather is SWDGE-descriptor-generation bound
    (~7.8 ns/row per queue, measured); 4 queues can sustain ~2 ns/row
    aggregate ONLY if 4 gather instructions stay in flight.  The v1
    design serialized on the AllGather prefix (225 us) and on per-
    superblock PSUM accumulation chains (avg 1.66 queues busy).  v2:
      * Each core computes the FULL h table itself, one chunk (25088
        rows) at a time, from a replicated x (51 MB bf16 HBM read,
        sequential, overlaps the gathers).  No AllGather, no barrier.
      * Phase C runs chunk-major: chunk c's 14 gather calls issue
        back-to-back on rotating queues while chunk c+1's projection
        streams on the scalar-engine HWDGE ring.
      * PSUM chains are per (chunk, block) only (S matmuls); partials
        accumulate into a bf16 SBUF accumulator via DVE, so PSUM tiles
        free immediately and gathers never wait on the scatter side.
  - Edges bucketed by (dst block of 128, h-chunk); groups of 128 edge
    slots padded per-call (S per (chunk, superblock)).  Per-edge h rows
    fetched with SWDGE dma_gather from HBM (256B bf16 rows); the
    weighted segment-sum is one matmul per 128-edge group with a HOST-
    precomputed edge_weight-scaled dst one-hot streamed from HBM.
  - single_packet=True crashes the device (measured); keep False.
"""

import os
import sys
from contextlib import ExitStack

import numpy as np

for _p in ("/opt/trn_rl_repo",):
    if _p not in sys.path and os.path.isdir(_p):
        sys.path.insert(0, _p)

import ml_dtypes  # noqa: E402

import concourse.bass as bass  # noqa: E402
import concourse.mybir as mybir  # noqa: E402
import concourse.tile as tile  # noqa: E402
from concourse import bacc, library_config  # noqa: E402
from concourse.bass_utils import run_bass_kernel_spmd  # noqa: E402

N_CORES = 8
P = 128
D_IN = 256
D_OUT = 128
NCHUNK = 4  # h-table chunks (int16 gather-index limit: 25088 <= 32767)
TS = 14  # projection row-tiles per strip

BF16 = ml_dtypes.bfloat16


def make_cfg(n_nodes: int) -> dict:
    assert n_nodes % N_CORES == 0
    r0 = n_nodes // N_CORES
    r = ((r0 + P - 1) // P) * P
    nb = r // P
    sb = 1
    for cand in (7, 8, 6, 5, 4, 9, 10, 3, 2, 14, 1):
        if nb % cand == 0:
            sb = cand
            break
    ntot = N_CORES * r
    ch = ntot // NCHUNK
    assert ch <= 32767, f"chunk rows {ch} exceed int16 index range"
    assert ch % P == 0
    return dict(
        n_nodes=n_nodes, R0=r0, R=r, NB=nb, SB=sb, NSB=nb // sb,
        NTOT=ntot, CH=ch, TPC=ch // P,
    )


ABLATE = os.environ.get("GNN_ABLATE", "")


def build_bass(cfg: dict, S_arr: tuple):
    """Build the SPMD Bass program (same NEFF for all 8 cores).

    S_arr[c*NSB + sb] = 128-edge groups per (dst block, call) for the
    gather call of chunk c, superblock sb.
    """
    R, NB, SB, NSB = cfg["R"], cfg["NB"], cfg["SB"], cfg["NSB"]
    NTOT, CH, TPC = cfg["NTOT"], cfg["CH"], cfg["TPC"]
    NCALL = NCHUNK * NSB
    assert len(S_arr) == NCALL
    S_max = max(S_arr)
    # per-call group offsets (call order: chunk-major, then superblock)
    call_goff = []
    goff = 0
    for call in range(NCALL):
        call_goff.append(goff)
        goff += SB * S_arr[call]
    NG = goff  # total 128-edge groups per core
    TOT = NG * P  # total padded edge slots per core

    f32 = mybir.dt.float32
    bf16 = mybir.dt.bfloat16
    i16 = mybir.dt.int16

    nc = bacc.Bacc(
        "TRN2",
        target_bir_lowering=False,
        debug=False,
        num_devices=N_CORES,
        num_swdge_queues=4,
    )

    xT = nc.declare_dram_parameter("xT", [D_IN, NTOT], bf16, isOutput=False)
    Wp = nc.declare_dram_parameter("W", [D_IN, D_OUT], bf16, isOutput=False)
    idxp = nc.declare_dram_parameter("idx", [P, TOT // 16], i16, isOutput=False)
    onehp = nc.declare_dram_parameter("oneh", [P, TOT], bf16, isOutput=False)
    outp = nc.declare_dram_parameter("out", [R, D_OUT], f32, isOutput=True)

    h_full = nc.dram_tensor("h_full", [NTOT, D_OUT], bf16)

    DK = D_IN // P  # k-chunks for the projection matmul
    NSTRIP = TPC // TS  # projection strips per chunk
    # strips of chunk c+1 interleaved between phase-C calls of chunk c.
    # Mildly front-loaded (finish by ~call 9): fully even spread leaves
    # c+1's last h-writes at the boundary (~95us stall per chunk), while
    # hard front-loading (2/call for 7 calls) saturates HBM and starves
    # the latency-sensitive gather reads (measured net-worse).
    _spc_base = (NSTRIP + NSB - 1) // NSB
    STRIP_PATTERN = []
    rem = NSTRIP
    for i in range(NSB):
        n = min(rem, 2 * _spc_base if i < 3 else _spc_base)
        STRIP_PATTERN.append(n)
        rem -= n
    assert rem == 0, (NSTRIP, STRIP_PATTERN)
    # per-chunk idx-tile column counts (whole chunk preloaded in one DMA)
    chunk_i16 = [
        sum(SB * S_arr[c * NSB + sb] * 8 for sb in range(NSB))
        for c in range(NCHUNK)
    ]
    CI_max = max(chunk_i16)

    with tile.TileContext(nc) as tc, ExitStack() as ctx:
        const = ctx.enter_context(tc.tile_pool(name="const", bufs=1))
        xpool = ctx.enter_context(tc.tile_pool(name="xp", bufs=2))
        hstage = ctx.enter_context(tc.tile_pool(name="hst", bufs=2))
        psum = ctx.enter_context(tc.tile_pool(name="ps", bufs=8, space="PSUM"))
        gpool = ctx.enter_context(tc.tile_pool(name="gat", bufs=5))
        ipool = ctx.enter_context(tc.tile_pool(name="idxp", bufs=2))
        opool = ctx.enter_context(tc.tile_pool(name="onehs", bufs=2))
        accp = ctx.enter_context(tc.tile_pool(name="accp", bufs=1))
        otp = ctx.enter_context(tc.tile_pool(name="otp", bufs=2))

        nc.gpsimd.load_library(library_config.mlp)

        w_t = const.tile([P, DK, P], bf16)
        for k in range(DK):
            nc.scalar.dma_start(out=w_t[:, k, :], in_=Wp[k * P : (k + 1) * P, :])

        acc = accp.tile([P, NB, P], bf16)

        def emit_strip(c, s_):
            """One projection strip: h_full[rows of strip s_ of chunk c]."""
            col0 = c * CH + s_ * TS * P
            xk = []
            for k in range(DK):
                xkt = xpool.tile([P, TS * P], bf16, tag=f"x{k}", name=f"x{k}")
                nc.scalar.dma_start(
                    out=xkt[:],
                    in_=xT[k * P : (k + 1) * P, col0 : col0 + TS * P],
                )
                xk.append(xkt)
            hst = hstage.tile([P, TS, P], bf16, tag="hst")
            for t in range(TS):
                ps = psum.tile([P, P], f32, tag="ps")
                for k in range(DK):
                    nc.tensor.matmul(
                        ps[:],
                        xk[k][:, t * P : (t + 1) * P],
                        w_t[:, k, :],
                        start=(k == 0),
                        stop=(k == DK - 1),
                    )
                nc.scalar.copy(out=hst[:, t, :], in_=ps[:])
            nc.scalar.dma_start(
                out=h_full[col0 : col0 + TS * P, :].rearrange(
                    "(t p) f -> p t f", p=P
                ),
                in_=hst[:],
            )

        # chunk 0 projection up front; chunk c+1's strips interleave with
        # chunk c's phase-C calls so TensorE never sees a projection wall.
        if "noproj" not in ABLATE:
            for s_ in range(NSTRIP):
                emit_strip(0, s_)

        for c in range(NCHUNK):
            next_strip = 0
            it = None
            for sb in range(NSB if "nophasec" not in ABLATE else 0):
                call = c * NSB + sb
                S = S_arr[call]
                NGC = SB * S
                NI = NGC * P
                goff = call_goff[call]
                if sb == 0:
                    # whole-chunk idx preload: gathers never wait on the
                    # per-call DMA FIFO.
                    it = ipool.tile([P, CI_max], i16, tag="it")
                    c0_16 = call_goff[c * NSB] * 8
                    nc.sync.dma_start(
                        out=it[:, : chunk_i16[c]],
                        in_=idxp[:, c0_16 : c0_16 + chunk_i16[c]],
                    )
                ioff = goff * 8 - call_goff[c * NSB] * 8
                gt = gpool.tile([P, SB * S_max, P], bf16, tag="gt")
                if "nogather" in ABLATE:
                    nc.vector.memset(gt[:], 0.0)
                else:
                    # split the call's gather into two half-calls on
                    # queues (q, q+2) — lands the halves on different
                    # SWDGE descgen contexts for parallel generation
                    G1 = (NGC + 1) // 2
                    N1 = G1 * P
                    nc.gpsimd.dma_gather(
                        gt[:, :G1, :],
                        h_full[c * CH : (c + 1) * CH, :],
                        it[:, ioff : ioff + N1 // 16],
                        N1,
                        N1,
                        P,
                        transpose=False,
                        single_packet=False,
                        queue_num=(2 * call) % 4,
                    )
                    nc.gpsimd.dma_gather(
                        gt[:, G1:NGC, :],
                        h_full[c * CH : (c + 1) * CH, :],
                        it[:, ioff + N1 // 16 : ioff + NI // 16],
                        NI - N1,
                        NI - N1,
                        P,
                        transpose=False,
                        single_packet=False,
                        queue_num=(2 * call + 1) % 4,
                    )
                oh = opool.tile([P, SB * S_max, P], bf16, tag="oh")
                nc.sync.dma_start(
                    out=oh[:, :NGC, :],
                    in_=onehp[:, goff * P : goff * P + NI],
                )
                ot = (
                    otp.tile([P, SB, P], f32, tag="ot", name=f"ot_{sb}")
                    if c == NCHUNK - 1
                    else None
                )
                for b in range(SB):
                    ps = psum.tile([P, P], f32, tag="ps")
                    for j in range(S):
                        g = b * S + j
                        nc.tensor.matmul(
                            ps[:],
                            oh[:, g, :],
                            gt[:, g, :],
                            start=(j == 0),
                            stop=(j == S - 1),
                        )
                    blk = sb * SB + b
                    if c == 0:
                        nc.vector.tensor_copy(acc[:, blk, :], ps[:])
                    elif c < NCHUNK - 1:
                        nc.vector.tensor_add(
                            acc[:, blk, :], acc[:, blk, :], ps[:]
                        )
                    else:
                        nc.vector.tensor_add(
                            ot[:, b, :], acc[:, blk, :], ps[:]
                        )
                if c == NCHUNK - 1:
                    nc.sync.dma_start(
                        out=outp[sb * SB * P : (sb + 1) * SB * P, :].rearrange(
                            "(b p) f -> p b f", p=P
                        ),
                        in_=ot[:],
                    )
                # interleave next chunk's projection strips
                if c + 1 < NCHUNK and "noproj" not in ABLATE:
                    for _ in range(STRIP_PATTERN[sb]):
                        if next_strip < NSTRIP:
                            emit_strip(c + 1, next_strip)
                            next_strip += 1
            if c + 1 < NCHUNK and "noproj" not in ABLATE:
                while next_strip < NSTRIP:
                    emit_strip(c + 1, next_strip)
                    next_strip += 1

        if "nophasec" in ABLATE:
            zt = otp.tile([P, SB, P], f32, tag="ot")
            nc.vector.memset(zt[:], 0.0)
            for sb in range(NSB):
                nc.sync.dma_start(
                    out=outp[sb * SB * P : (sb + 1) * SB * P, :].rearrange(
                        "(b p) f -> p b f", p=P
                    ),
                    in_=zt[:],
                )

    nc.compile()
    return nc


def host_prep(x, W, edge_src, edge_dst, edge_weight, cfg):
    """Shard + stage inputs. Returns (in_maps, S_arr)."""
    R0, R, NB, SB, NSB = cfg["R0"], cfg["R"], cfg["NB"], cfg["SB"], cfg["NSB"]
    NTOT = cfg["NTOT"]
    x = np.asarray(x, dtype=np.float32)
    W = np.asarray(W, dtype=np.float32)
    edge_src = np.asarray(edge_src, dtype=np.int64)
    edge_dst = np.asarray(edge_dst, dtype=np.int64)
    edge_weight = np.asarray(edge_weight, dtype=np.float32)
    NCALL = NCHUNK * NSB

    # h_full row layout: row = m_s*R + l_s (per-core shards padded to R);
    # chunk c = row // CH; chunk-local index fits int16.
    m_s = edge_src // R0
    l_s = edge_src - m_s * R0
    src_chunk = (m_s // 2).astype(np.int64)
    idx16_all = ((m_s % 2) * R + l_s).astype(np.int64)

    core_of = edge_dst // R0
    per_core = []
    for m in range(N_CORES):
        sel = core_of == m
        d = edge_dst[sel] - m * R0
        w = edge_weight[sel]
        b = d // P
        dstl = (d % P).astype(np.int64)
        c = src_chunk[sel]
        lidx = idx16_all[sel].astype(np.int16)
        key = (b * NCHUNK + c).astype(np.int64)
        counts = np.bincount(key, minlength=NB * NCHUNK)
        per_core.append((b, c, dstl, lidx, w, key, counts))

    # S per call (chunk, superblock): max bucket count over cores and
    # blocks in the superblock for that chunk.
    all_counts = np.stack([pc[6] for pc in per_core]).reshape(
        N_CORES, NB, NCHUNK
    )
    cmax = all_counts.max(axis=0)  # [NB, NCHUNK]
    S_list = []
    for c in range(NCHUNK):
        for sb in range(NSB):
            m = cmax[sb * SB : (sb + 1) * SB, c].max()
            S_list.append(max(1, int((m + P - 1) // P)))
    S_arr = np.array(S_list, dtype=np.int64)
    call_goff = np.zeros(NCALL, dtype=np.int64)
    np.cumsum(SB * S_arr[:-1], out=call_goff[1:])
    NG = int(SB * S_arr.sum())
    TOT = NG * P

    # replicated x, padded per-core-shard layout, transposed, bf16
    x_pad = np.zeros((NTOT, D_IN), dtype=BF16)
    for m in range(N_CORES):
        x_pad[m * R : m * R + R0] = x[m * R0 : (m + 1) * R0].astype(BF16)
    xT_full = np.ascontiguousarray(x_pad.T)
    W_bf = W.astype(BF16)

    in_maps = []
    for m in range(N_CORES):
        b, c, dstl, lidx, w, key, counts = per_core[m]
        # Sort edges by (block, chunk) bucket, then by gather idx within
        # each bucket (HBM page locality).
        order = np.lexsort((lidx, key))
        key_s = key[order]
        starts = np.zeros(NB * NCHUNK + 1, dtype=np.int64)
        np.cumsum(counts, out=starts[1:])
        rank = np.arange(len(key_s)) - starts[key_s]
        bb = b[order]
        cc = c[order]
        call_of = cc * NSB + bb // SB
        slot = (
            call_goff[call_of] * P
            + (bb % SB) * S_arr[call_of] * P
            + rank
        )

        idx_stream = np.zeros(TOT, dtype=np.int16)
        idx_stream[slot] = lidx[order]
        # per-call 16-partition wrap, replicated to 128 partitions
        idx_wrapped = np.empty((P, TOT // 16), dtype=np.int16)
        for call in range(NCALL):
            o = call_goff[call] * P
            ni = SB * int(S_arr[call]) * P
            seg = idx_stream[o : o + ni].reshape(-1, 16).T  # [16, ni/16]
            idx_wrapped[:, o // 16 : (o + ni) // 16] = np.tile(seg, (8, 1))

        # Host-precomputed weighted one-hot: edge at slot s (group g =
        # s//128, lane e = s%128): oneh[e, g*128 + dstl] = w.
        oneh = np.zeros((P, TOT), dtype=BF16)
        oneh[slot % P, (slot // P) * P + dstl[order]] = w[order].astype(BF16)

        in_maps.append(
            {
                "xT": xT_full,
                "W": W_bf,
                "idx": idx_wrapped,
                "oneh": oneh,
            }
        )
    return in_maps, tuple(S_list)


_BUILD_CACHE: dict = {}


def run(x, W, edge_src, edge_dst, edge_weight, trace=False, trace_kwargs=None):
    n_nodes = x.shape[0]
    cfg = make_cfg(n_nodes)
    in_maps, S_arr = host_prep(x, W, edge_src, edge_dst, edge_weight, cfg)
    key = (n_nodes, S_arr)
    if key not in _BUILD_CACHE:
        _BUILD_CACHE[key] = build_bass(cfg, S_arr)
    nc = _BUILD_CACHE[key]
    res = run_bass_kernel_spmd(
        nc,
        in_maps,
        core_ids=list(range(N_CORES)),
        trace=trace,
        **(trace_kwargs or {}),
    )
    R0, R = cfg["R0"], cfg["R"]
    out = np.concatenate(
        [np.asarray(res.results[m]["out"])[:R0] for m in range(N_CORES)], axis=0
    )
    return out, res


def kernel(**inputs) -> np.ndarray:
    out, _ = run(
        inputs["x"],
        inputs["W"],
        inputs["edge_src"],
        inputs["edge_dst"],
        inputs["edge_weight"],
        trace=False,
    )
    return out
